# revision 9
# baseline (speedup 1.0000x reference)
"""Trainium2 Bass kernel for nn_CustomLoss_54400055771232.

Computes, over full inputs:
    mse   = mean_c (preds - targets)^2                      # [B, T]
    w     = nee_qc * igbp_table[igbp] * koppen_table[koppen]
    bal   = (preds[..2] + preds[..0] - preds[..1])^2        # [B, T]
    out   = mean_bt(mse * w + ALPHA * bal)                  # scalar

Strategy: pure data-parallel over B across 8 NeuronCores. The key
bottleneck in the naive formulation is the 16-class igbp weighted
binning: DVE scalar_tensor_tensor has NO fast perf modes (always 1x),
so 16 masked passes over [B,T] cost ~100us/core. Instead, the host
re-orders each partition row's 5840 elements by combined class
ci = 5*igbp + koppen into 80 fixed-size buckets (PAD=78 each, zero
padded), with the rare bucket overflow spilled to a 256-wide "misc"
region. The permutation is lossless; padding elements have
preds=targets=qc=0 so they contribute exactly 0 to every sum. On
device the per-class sums then become plain fixed-range reductions
(one tensor_reduce per tile), and only the ~2% spill elements take the
slow masked-stt path. Host applies the 80-entry weight table
(igbp_table x koppen_table outer product) to the bucket sums in f64 --
linear post-processing, same as applying the mean.

Data is bf16 (halves HBM traffic, unlocks DVE 2x tensor_tensor mode);
layout is tile-major + channel-major so the sum over C=6 uses
contiguous step-1 slices (DVE 2x) instead of strided GPSIMD ops.
Squares run on the otherwise-idle ScalarE, chunked per channel-pair so
the DVE csum folds interleave. The misc region lives in tile 0 so its
serial stt chain overlaps the streaming phase. No GPSIMD (it contends
with DVE for SBUF ports).
"""

import sys

if "/opt/trn_rl_repo" not in sys.path:
    sys.path.insert(0, "/opt/trn_rl_repo")

import numpy as np
import ml_dtypes

import concourse.bass as bass
import concourse.bacc as bacc
import concourse.tile as tile
from concourse import mybir
from concourse.bass_utils import run_bass_kernel_spmd

# Problem constants (hardcoded per harness contract).
B, T, C = 16384, 365, 6
N_IGBP, N_KOPPEN = 16, 5
ALPHA = 0.1
N_CORES = 8

B_CORE = B // N_CORES            # 2048
P = 128                          # partitions
FP = B_CORE * T // P             # 5840 real bt elems per partition

NB = N_IGBP * N_KOPPEN           # 80 combined classes
PAD = 78                         # bucket capacity (seed-0 max spill 188)
NBUCK = NB * PAD                 # 6240 bucketed cols
NT = 8                           # tiles
# graded bucket counts per tile: small first tile so compute starts as
# soon as possible, small last tile so the drain tail is short
BPTS = [2, 8, 12, 13, 13, 13, 13, 6]
BOFF = [0, 2, 10, 22, 35, 48, 61, 74]

f32 = mybir.dt.float32
bf16 = mybir.dt.bfloat16

AF = mybir.ActivationFunctionType
OP = mybir.AluOpType
AX = mybir.AxisListType

_CACHE = {}


def _build(misc):
    lay = NBUCK + misc
    # tile 0 holds the misc region + its buckets; host cols
    # [0, misc) = misc, then NB*PAD bucketed
    fts = [BPTS[t] * PAD + (misc if t == 0 else 0) for t in range(NT)]
    offs = np.cumsum([0] + fts).tolist()

    nc = bacc.Bacc("TRN2", target_bir_lowering=False, debug=False,
                   num_devices=N_CORES)

    preds = nc.dram_tensor("preds", [P, lay * C], bf16, kind="ExternalInput").ap()
    targs = nc.dram_tensor("targs", [P, lay * C], bf16, kind="ExternalInput").ap()
    qc = nc.dram_tensor("qc", [P, lay], bf16, kind="ExternalInput").ap()
    igm = nc.dram_tensor("igm", [P, misc], bf16, kind="ExternalInput").ap()
    kpm = nc.dram_tensor("kpm", [P, misc], bf16, kind="ExternalInput").ap()
    # koppen_table values replicated per partition (per-partition scalar APs
    # so table values stay runtime inputs, no recompile per call)
    coef = nc.dram_tensor("coef", [P, N_KOPPEN], f32, kind="ExternalInput").ap()

    bsum_o = nc.dram_tensor("bsum", [P, NB], f32, kind="ExternalOutput").ap()
    macc_o = nc.dram_tensor("macc", [P, N_IGBP], f32, kind="ExternalOutput").ap()
    bal_o = nc.dram_tensor("bal", [P, NT], f32, kind="ExternalOutput").ap()

    with tile.TileContext(nc) as tc:
        with (
            tc.tile_pool(name="big", bufs=3) as big,     # streamed BTC tiles
            tc.tile_pool(name="work", bufs=2) as work,   # per-tile scratch
            tc.tile_pool(name="bt", bufs=1) as bt,       # [B,T]-wide tensors
            tc.tile_pool(name="accs", bufs=1) as accs,   # persistent outputs
        ):
            coef_t = accs.tile([P, N_KOPPEN], f32)
            nc.sync.dma_start(coef_t[:], coef[:])
            t2ap = lambda l: coef_t[:, l: l + 1]
            bsum_t = accs.tile([P, NB], f32)
            macc_t = accs.tile([P, N_IGBP], f32)
            bal_t = accs.tile([P, NT], f32)

            q_t = bt.tile([P, lay], bf16)
            igm_t = bt.tile([P, misc], bf16)
            kpm_t = bt.tile([P, misc], bf16)
            z_full = bt.tile([P, lay], bf16)

            def misc_stage():
                # w2 = koppen_table[kpm] via 5 one-hot ts ops, then 16
                # igbp-masked 1x stt passes over z*w2 (tiny: misc cols)
                w2m_t = bt.tile([P, misc], bf16)
                ha_t = bt.tile([P, misc], bf16)
                hb_t = bt.tile([P, misc], bf16)
                nc.vector.tensor_scalar(ha_t[:], kpm_t[:], 0.0, t2ap(0),
                                        OP.is_equal, OP.mult)
                nc.vector.tensor_scalar(hb_t[:], kpm_t[:], 1.0, t2ap(1),
                                        OP.is_equal, OP.mult)
                nc.vector.tensor_add(w2m_t[:], ha_t[:], hb_t[:])
                for l in range(2, N_KOPPEN):
                    h = ha_t if l % 2 == 0 else hb_t
                    nc.vector.tensor_scalar(h[:], kpm_t[:], float(l), t2ap(l),
                                            OP.is_equal, OP.mult)
                    nc.vector.tensor_add(w2m_t[:], w2m_t[:], h[:])
                vm_t = bt.tile([P, misc], bf16)
                nc.vector.tensor_mul(vm_t[:], z_full[:, 0:misc], w2m_t[:])
                sc_t = bt.tile([P, misc], bf16)
                for k in range(N_IGBP):
                    nc.vector.scalar_tensor_tensor(
                        sc_t[:], igm_t[:], float(k), vm_t[:],
                        OP.is_equal, OP.mult,
                        accum_out=macc_t[:, k: k + 1])

            for t in range(NT):
                ft = fts[t]
                o = offs[t]
                p_t = big.tile([P, ft * C], bf16, tag="p")
                nc.sync.dma_start(p_t[:], preds[:, o * C: o * C + ft * C])
                g_t = big.tile([P, ft * C], bf16, tag="tg")
                nc.sync.dma_start(g_t[:], targs[:, o * C: o * C + ft * C])
                # qc chunk (and misc indices) stream in behind the big tiles
                nc.sync.dma_start(q_t[:, o: o + ft], qc[:, o: o + ft])
                if t == 0:
                    nc.sync.dma_start(igm_t[:], igm[:])
                    nc.sync.dma_start(kpm_t[:], kpm[:])

                # balance: e = (p0 - p1) + p2 on contiguous channel slices
                e_t = work.tile([P, ft], bf16, tag="e")
                nc.vector.tensor_sub(e_t[:], p_t[:, 0:ft], p_t[:, ft:2 * ft])
                nc.vector.tensor_add(e_t[:], e_t[:], p_t[:, 2 * ft:3 * ft])
                e2_t = work.tile([P, ft], bf16, tag="e2")
                nc.scalar.activation(e2_t[:], e_t[:], AF.Square,
                                     accum_out=bal_t[:, t: t + 1])

                # d = p - t in place into targets tile (DVE bf16 2x)
                nc.vector.tensor_sub(g_t[:], p_t[:], g_t[:])
                # square halves on ScalarE, then s = sum over C via
                # contiguous channel-block adds (all DVE 2x)
                nc.scalar.activation(g_t[:, 0:3 * ft], g_t[:, 0:3 * ft],
                                     AF.Square)
                nc.scalar.activation(g_t[:, 3 * ft:6 * ft],
                                     g_t[:, 3 * ft:6 * ft], AF.Square)
                u_t = work.tile([P, 3 * ft], bf16, tag="u")
                nc.vector.tensor_add(u_t[:], g_t[:, 0:3 * ft],
                                     g_t[:, 3 * ft:6 * ft])
                r_t = work.tile([P, ft], bf16, tag="r")
                nc.vector.tensor_add(r_t[:], u_t[:, 0:ft], u_t[:, ft:2 * ft])
                sv = work.tile([P, ft], bf16, tag="s")
                nc.vector.tensor_add(sv[:], r_t[:], u_t[:, 2 * ft:3 * ft])

                # z = s * qc for this tile's cols
                zv = z_full[:, o: o + ft]
                nc.vector.tensor_mul(zv[:], sv[:], q_t[:, o: o + ft])

                # bucket sums for this tile's buckets
                bo, bn = BOFF[t], BPTS[t]
                zb = z_full[:, misc + bo * PAD: misc + (bo + bn) * PAD]
                zb3 = zb.rearrange("p (b e) -> p b e", b=bn)
                nc.vector.tensor_reduce(
                    bsum_t[:, bo: bo + bn], zb3[:],
                    axis=AX.X, op=OP.add)
                nc.sync.dma_start(bsum_o[:, bo: bo + bn],
                                  bsum_t[:, bo: bo + bn])

                if t == 0:
                    misc_stage()
                    nc.sync.dma_start(macc_o[:], macc_t[:])

            nc.sync.dma_start(bal_o[:], bal_t[:])

    nc.finalize()
    return nc


def _run_spmd(in_maps, misc, trace=False, trace_kwargs=None):
    if misc not in _CACHE:
        _CACHE[misc] = _build(misc)
    return run_bass_kernel_spmd(_CACHE[misc], in_maps, list(range(N_CORES)),
                                trace=trace, **(trace_kwargs or {}))


def _pack_core(preds6, targs6, qcv, igv, kpv, misc):
    """Bucket-sort one core's [P, FP] rows by ci=5*ig+kp into the padded
    layout (misc region first, then NB*PAD bucket cols). Returns packed
    preds/targs [P, lay*C] (tile-major, channel-major), qc [P, lay],
    ig/kp misc [P, misc]."""
    lay = NBUCK + misc
    ci = igv * N_KOPPEN + kpv                              # [P, FP]
    order = np.argsort(ci, axis=1, kind="stable")
    sci = np.take_along_axis(ci, order, axis=1)
    cnt = np.zeros((P, NB), np.int64)
    rows2d = np.broadcast_to(np.arange(P)[:, None], (P, FP))
    np.add.at(cnt, (rows2d.ravel(), ci.ravel()), 1)
    start = np.zeros((P, NB), np.int64)
    start[:, 1:] = np.cumsum(cnt, axis=1)[:, :-1]
    rank = np.arange(FP)[None, :] - np.take_along_axis(start, sci, axis=1)
    spill = rank >= PAD
    mrank = np.cumsum(spill, axis=1) - 1
    max_spill = int(mrank[:, -1].max()) + 1 if spill.any() else 0
    if max_spill > misc:
        raise OverflowError(max_spill)
    dest = np.where(spill, mrank,
                    misc + sci * PAD + np.minimum(rank, PAD - 1))

    ridx = rows2d
    bf = ml_dtypes.bfloat16

    qb = np.zeros((P, lay), qcv.dtype)
    qb[ridx, dest] = np.take_along_axis(qcv, order, axis=1)

    out6 = np.zeros((P, lay, C), preds6.dtype)
    tg6 = np.zeros((P, lay, C), targs6.dtype)
    o3 = order[:, :, None]
    out6[ridx, dest] = np.take_along_axis(preds6, o3, axis=1)
    tg6[ridx, dest] = np.take_along_axis(targs6, o3, axis=1)

    fts = [BPTS[t] * PAD + (misc if t == 0 else 0) for t in range(NT)]
    offs = np.cumsum([0] + fts).tolist()

    def tilemajor(x6):
        blocks = [
            np.ascontiguousarray(
                x6[:, o: o + ft, :].transpose(0, 2, 1)).reshape(P, C * ft)
            for o, ft in zip(offs, fts)
        ]
        return np.concatenate(blocks, axis=1).astype(bf)

    igm = np.full((P, misc), 255.0, np.float32)
    kpm = np.zeros((P, misc), np.float32)
    sig = np.take_along_axis(igv, order, axis=1)
    skp = np.take_along_axis(kpv, order, axis=1)
    igm[ridx[spill], mrank[spill]] = sig[spill]
    kpm[ridx[spill], mrank[spill]] = skp[spill]

    return {
        "preds": tilemajor(out6),
        "targs": tilemajor(tg6),
        "qc": qb.astype(bf),
        "igm": igm.astype(bf),
        "kpm": kpm.astype(bf),
    }


def make_in_maps(preds, targets, nee_qc, igbp, koppen, igbp_table,
                 koppen_table, misc=224):
    preds = np.asarray(preds, np.float32)
    targets = np.asarray(targets, np.float32)
    nee_qc = np.asarray(nee_qc, np.float32)
    igbp = np.asarray(igbp, np.int64)
    koppen = np.asarray(koppen, np.int64)

    t2 = np.asarray(koppen_table, np.float32)
    coef_np = np.tile(t2[None, :], (P, 1))

    in_maps = []
    for m in range(N_CORES):
        b0, b1 = m * B_CORE, (m + 1) * B_CORE
        mp = _pack_core(
            preds[b0:b1].reshape(P, FP, C),
            targets[b0:b1].reshape(P, FP, C),
            nee_qc[b0:b1].reshape(P, FP),
            igbp[b0:b1].reshape(P, FP),
            koppen[b0:b1].reshape(P, FP),
            misc,
        )
        mp["coef"] = coef_np
        in_maps.append(mp)
    return in_maps


def finish(res, igbp_table, koppen_table):
    t1 = np.asarray(igbp_table, np.float64)
    t2 = np.asarray(koppen_table, np.float64)
    w12 = np.outer(t1, t2).reshape(NB)           # bucket ci = 5*ig + kp
    mse_sum = 0.0
    bal_sum = 0.0
    for m in range(N_CORES):
        bs = res.results[m]["bsum"].astype(np.float64)    # [P, NB]
        ma = res.results[m]["macc"].astype(np.float64)    # [P, N_IGBP]
        bl = res.results[m]["bal"].astype(np.float64)     # [P, NT]
        mse_sum += float((bs.sum(axis=0) * w12).sum())
        mse_sum += float((ma.sum(axis=0) * t1).sum())
        bal_sum += float(bl.sum())
    total = (mse_sum / C + ALPHA * bal_sum) / (B * T)
    return np.float32(total)


def kernel(preds, targets, nee_qc, igbp, koppen, igbp_table, koppen_table):
    for misc in (224, 1024, 4096):
        try:
            in_maps = make_in_maps(preds, targets, nee_qc, igbp, koppen,
                                   igbp_table, koppen_table, misc=misc)
        except OverflowError:
            continue
        res = _run_spmd(in_maps, misc)
        return finish(res, igbp_table, koppen_table)
    raise RuntimeError("bucket spill exceeded all misc capacities")


# revision 16
# speedup vs baseline: 1.1119x; 1.1119x over previous
"""Trainium2 Bass kernel for nn_CustomLoss_54400055771232.

Computes, over full inputs:
    mse   = mean_c (preds - targets)^2                      # [B, T]
    w     = nee_qc * igbp_table[igbp] * koppen_table[koppen]
    bal   = (preds[..2] + preds[..0] - preds[..1])^2        # [B, T]
    out   = mean_bt(mse * w + ALPHA * bal)                  # scalar

Strategy: pure data-parallel over B across 8 NeuronCores. The key
bottleneck in the naive formulation is the 16-class igbp weighted
binning: DVE scalar_tensor_tensor has NO fast perf modes (always 1x),
so 16 masked passes over [B,T] cost ~100us/core. Instead, the host
re-orders each partition row's 5840 elements by combined class
ci = 5*igbp + koppen into 80 fixed-size buckets (PAD=78 each, zero
padded), with the rare bucket overflow spilled to a small "misc"
region. The permutation is lossless; padding elements have
preds=targets=qc=0 so they contribute exactly 0 to every sum. On
device the per-class sums then become plain fixed-range reductions
(one tensor_reduce per tile), and only the ~2% spill elements take the
slow masked-stt path. Host applies the 80-entry weight table
(igbp_table x koppen_table outer product) to the bucket sums in f64 --
linear post-processing, same as applying the mean.

Data is bf16 (halves HBM traffic, unlocks DVE 2x tensor_tensor mode);
layout is tile-major + channel-major so the sum over C=6 uses
contiguous step-1 slices (DVE 2x) instead of strided GPSIMD ops.
Squares run on the otherwise-idle ScalarE. The misc region lives in
tile 0 so its serial stt chain overlaps the streaming phase. No GPSIMD
(it contends with DVE for SBUF ports). Each tile's preds/targets/qc
(+ tile 0: misc index arrays and the koppen table) are packed into ONE
interleaved DRAM block so each tile costs a single ~650ns Sync-engine
DMA dispatch instead of three-plus (dispatches are serial and were
~20% of runtime); outputs are coalesced at the end for the same
reason.
"""

import sys

if "/opt/trn_rl_repo" not in sys.path:
    sys.path.insert(0, "/opt/trn_rl_repo")

import numpy as np
import ml_dtypes

import concourse.bass as bass
import concourse.bacc as bacc
import concourse.tile as tile
from concourse import mybir
from concourse.bass_utils import run_bass_kernel_spmd

# Problem constants (hardcoded per harness contract).
B, T, C = 16384, 365, 6
N_IGBP, N_KOPPEN = 16, 5
ALPHA = 0.1
N_CORES = 8

B_CORE = B // N_CORES            # 2048
P = 128                          # partitions
FP = B_CORE * T // P             # 5840 real bt elems per partition

NB = N_IGBP * N_KOPPEN           # 80 combined classes
PAD = 78                         # bucket capacity (seed-0 max row spill 188)
NBUCK = NB * PAD                 # 6240 bucketed cols
NT = 8                           # tiles
# graded bucket counts per tile: small first tile so compute starts as
# soon as possible, small last tile so the drain tail is short
BPTS = [2, 8, 12, 13, 13, 13, 13, 6]
BOFF = [0, 2, 10, 22, 35, 48, 61, 74]
CPAD = 12                        # coef slot: 5 f32 as 10 bf16 slots + pad

f32 = mybir.dt.float32
bf16 = mybir.dt.bfloat16

AF = mybir.ActivationFunctionType
OP = mybir.AluOpType
AX = mybir.AxisListType

_CACHE = {}


def _geom(misc):
    lay = NBUCK + misc
    # global column layout: tile 0 = [misc | its buckets], then tiles 1..
    fts = [BPTS[t] * PAD + (misc if t == 0 else 0) for t in range(NT)]
    offs = np.cumsum([0] + fts).tolist()
    # per-tile packed block: [p C*ft | t C*ft | q ft] (+ tile0: igm, kpm, coef)
    bss = [13 * fts[t] + (2 * misc + CPAD if t == 0 else 0) for t in range(NT)]
    boffs = np.cumsum([0] + bss).tolist()
    return lay, fts, offs, bss, boffs


def _build(misc):
    lay, fts, offs, bss, boffs = _geom(misc)

    nc = bacc.Bacc("TRN2", target_bir_lowering=False, debug=False,
                   num_devices=N_CORES)

    blk = nc.dram_tensor("blk", [P, boffs[-1]], bf16, kind="ExternalInput").ap()
    out_o = nc.dram_tensor("out", [P, NB + N_IGBP + NT], f32,
                           kind="ExternalOutput").ap()

    with tile.TileContext(nc) as tc:
        with (
            tc.tile_pool(name="big", bufs=4) as big,     # streamed packed tiles
            tc.tile_pool(name="work", bufs=2) as work,   # per-tile scratch
            tc.tile_pool(name="bt", bufs=1) as bt,       # misc-stage tensors
            tc.tile_pool(name="accs", bufs=1) as accs,   # persistent outputs
        ):
            out_t = accs.tile([P, NB + N_IGBP + NT], f32)
            bsum_t = out_t[:, 0:NB]
            macc_t = out_t[:, NB:NB + N_IGBP]
            bal_t = out_t[:, NB + N_IGBP:]
            z_full = bt.tile([P, lay], bf16)

            def misc_stage(b0):
                # tile-0 block extras
                igm = b0[:, 13 * fts[0]: 13 * fts[0] + misc]
                kpm = b0[:, 13 * fts[0] + misc: 13 * fts[0] + 2 * misc]
                cof = b0[:, 13 * fts[0] + 2 * misc:
                         13 * fts[0] + 2 * misc + CPAD].bitcast(f32)
                t2ap = lambda l: cof[:, l: l + 1]
                # w2 = koppen_table[kpm] via 5 one-hot ts ops, then 16
                # igbp-masked 1x stt passes over z*w2 (tiny: misc cols)
                w2m_t = bt.tile([P, misc], bf16)
                ha_t = bt.tile([P, misc], bf16)
                hb_t = bt.tile([P, misc], bf16)
                nc.vector.tensor_scalar(ha_t[:], kpm[:], 0.0, t2ap(0),
                                        OP.is_equal, OP.mult)
                nc.vector.tensor_scalar(hb_t[:], kpm[:], 1.0, t2ap(1),
                                        OP.is_equal, OP.mult)
                nc.vector.tensor_add(w2m_t[:], ha_t[:], hb_t[:])
                for l in range(2, N_KOPPEN):
                    h = ha_t if l % 2 == 0 else hb_t
                    nc.vector.tensor_scalar(h[:], kpm[:], float(l), t2ap(l),
                                            OP.is_equal, OP.mult)
                    nc.vector.tensor_add(w2m_t[:], w2m_t[:], h[:])
                vm_t = bt.tile([P, misc], bf16)
                nc.vector.tensor_mul(vm_t[:], z_full[:, 0:misc], w2m_t[:])
                sc_t = bt.tile([P, misc], bf16)
                for k in range(N_IGBP):
                    nc.vector.scalar_tensor_tensor(
                        sc_t[:], igm[:], float(k), vm_t[:],
                        OP.is_equal, OP.mult,
                        accum_out=macc_t[:, k: k + 1])

            for t in range(NT):
                ft = fts[t]
                o = offs[t]
                b_t = big.tile([P, bss[t]], bf16, tag="b")
                if t == 0:
                    sp = 6 * ft
                    nc.sync.dma_start(b_t[:, 0:sp], blk[:, boffs[t]: boffs[t] + sp])
                    nc.sync.dma_start(b_t[:, sp:], blk[:, boffs[t] + sp: boffs[t + 1]])
                else:
                    nc.sync.dma_start(b_t[:], blk[:, boffs[t]: boffs[t + 1]])
                p_t = b_t[:, 0: 6 * ft]
                g_t = b_t[:, 6 * ft: 12 * ft]
                qv = b_t[:, 12 * ft: 13 * ft]

                # balance: e = (p0 - p1) + p2 on contiguous channel slices
                e_t = work.tile([P, ft], bf16, tag="e")
                nc.vector.tensor_sub(e_t[:], p_t[:, 0:ft], p_t[:, ft:2 * ft])
                nc.vector.tensor_add(e_t[:], e_t[:], p_t[:, 2 * ft:3 * ft])
                e2_t = work.tile([P, ft], bf16, tag="e2")
                nc.scalar.activation(e2_t[:], e_t[:], AF.Square,
                                     accum_out=bal_t[:, t: t + 1])

                # d = p - t in place into the targets half (DVE bf16 2x)
                nc.vector.tensor_sub(g_t[:], p_t[:], g_t[:])
                # square halves on ScalarE, then s = sum over C via
                # contiguous channel-block adds (all DVE 2x)
                nc.scalar.activation(g_t[:, 0:3 * ft], g_t[:, 0:3 * ft],
                                     AF.Square)
                nc.scalar.activation(g_t[:, 3 * ft:6 * ft],
                                     g_t[:, 3 * ft:6 * ft], AF.Square)
                u_t = work.tile([P, 3 * ft], bf16, tag="u")
                nc.vector.tensor_add(u_t[:], g_t[:, 0:3 * ft],
                                     g_t[:, 3 * ft:6 * ft])
                r_t = work.tile([P, ft], bf16, tag="r")
                nc.vector.tensor_add(r_t[:], u_t[:, 0:ft], u_t[:, ft:2 * ft])
                sv = work.tile([P, ft], bf16, tag="s")
                nc.vector.tensor_add(sv[:], r_t[:], u_t[:, 2 * ft:3 * ft])

                # z = s * qc for this tile's cols
                zv = z_full[:, o: o + ft]
                nc.vector.tensor_mul(zv[:], sv[:], qv[:])

                # bucket sums for this tile's buckets
                bo, bn = BOFF[t], BPTS[t]
                zb = z_full[:, misc + bo * PAD: misc + (bo + bn) * PAD]
                zb3 = zb.rearrange("p (b e) -> p b e", b=bn)
                nc.vector.tensor_reduce(
                    bsum_t[:, bo: bo + bn], zb3[:],
                    axis=AX.X, op=OP.add)

                if t == 0:
                    misc_stage(b_t)

            nc.sync.dma_start(out_o[:], out_t[:])

    nc.finalize()
    return nc


def _run_spmd(in_maps, misc, trace=False, trace_kwargs=None):
    if misc not in _CACHE:
        _CACHE[misc] = _build(misc)
    return run_bass_kernel_spmd(_CACHE[misc], in_maps, list(range(N_CORES)),
                                trace=trace, **(trace_kwargs or {}))


def _pack_core(preds6, targs6, qcv, igv, kpv, t2, misc):
    """Bucket-sort one core's [P, FP] rows by ci=5*ig+kp into the padded
    layout (misc region first, then NB*PAD bucket cols), then pack
    everything into a single per-tile-interleaved block array."""
    lay, fts, offs, bss, boffs = _geom(misc)
    ci = igv * N_KOPPEN + kpv                              # [P, FP]
    order = np.argsort(ci, axis=1, kind="stable")
    sci = np.take_along_axis(ci, order, axis=1)
    cnt = np.zeros((P, NB), np.int64)
    rows2d = np.broadcast_to(np.arange(P)[:, None], (P, FP))
    np.add.at(cnt, (rows2d.ravel(), ci.ravel()), 1)
    start = np.zeros((P, NB), np.int64)
    start[:, 1:] = np.cumsum(cnt, axis=1)[:, :-1]
    rank = np.arange(FP)[None, :] - np.take_along_axis(start, sci, axis=1)
    spill = rank >= PAD
    mrank = np.cumsum(spill, axis=1) - 1
    max_spill = int(mrank[:, -1].max()) + 1 if spill.any() else 0
    if max_spill > misc:
        raise OverflowError(max_spill)
    dest = np.where(spill, mrank,
                    misc + sci * PAD + np.minimum(rank, PAD - 1))

    ridx = rows2d
    bf = ml_dtypes.bfloat16

    qb = np.zeros((P, lay), qcv.dtype)
    qb[ridx, dest] = np.take_along_axis(qcv, order, axis=1)

    out6 = np.zeros((P, lay, C), preds6.dtype)
    tg6 = np.zeros((P, lay, C), targs6.dtype)
    o3 = order[:, :, None]
    out6[ridx, dest] = np.take_along_axis(preds6, o3, axis=1)
    tg6[ridx, dest] = np.take_along_axis(targs6, o3, axis=1)

    igm = np.full((P, misc), 255.0, np.float32)
    kpm = np.zeros((P, misc), np.float32)
    sig = np.take_along_axis(igv, order, axis=1)
    skp = np.take_along_axis(kpv, order, axis=1)
    igm[ridx[spill], mrank[spill]] = sig[spill]
    kpm[ridx[spill], mrank[spill]] = skp[spill]
    cof = np.zeros((P, CPAD // 2), np.float32)
    cof[:, :N_KOPPEN] = t2[None, :]
    cofb = cof.view(np.uint16).view(ml_dtypes.bfloat16)  # raw f32 bytes

    blocks = []
    for t, (o, ft) in enumerate(zip(offs, fts)):
        blocks.append(np.ascontiguousarray(
            out6[:, o: o + ft, :].transpose(0, 2, 1)).reshape(P, C * ft))
        blocks.append(np.ascontiguousarray(
            tg6[:, o: o + ft, :].transpose(0, 2, 1)).reshape(P, C * ft))
        blocks.append(qb[:, o: o + ft])
        if t == 0:
            blocks += [igm.astype(bf), kpm.astype(bf), cofb]
    return {"blk": np.concatenate(blocks, axis=1).astype(bf)}


def make_in_maps(preds, targets, nee_qc, igbp, koppen, igbp_table,
                 koppen_table, misc=224):
    preds = np.asarray(preds, np.float32)
    targets = np.asarray(targets, np.float32)
    nee_qc = np.asarray(nee_qc, np.float32)
    igbp = np.asarray(igbp, np.int64)
    koppen = np.asarray(koppen, np.int64)
    t2 = np.asarray(koppen_table, np.float32)

    in_maps = []
    for m in range(N_CORES):
        b0, b1 = m * B_CORE, (m + 1) * B_CORE
        in_maps.append(_pack_core(
            preds[b0:b1].reshape(P, FP, C),
            targets[b0:b1].reshape(P, FP, C),
            nee_qc[b0:b1].reshape(P, FP),
            igbp[b0:b1].reshape(P, FP),
            koppen[b0:b1].reshape(P, FP),
            t2, misc,
        ))
    return in_maps


def finish(res, igbp_table, koppen_table):
    t1 = np.asarray(igbp_table, np.float64)
    t2 = np.asarray(koppen_table, np.float64)
    w12 = np.outer(t1, t2).reshape(NB)           # bucket ci = 5*ig + kp
    mse_sum = 0.0
    bal_sum = 0.0
    for m in range(N_CORES):
        out = res.results[m]["out"].astype(np.float64)    # [P, NB+16+NT]
        bs = out[:, :NB]
        ma = out[:, NB:NB + N_IGBP]
        bl = out[:, NB + N_IGBP:]
        mse_sum += float((bs.sum(axis=0) * w12).sum())
        mse_sum += float((ma.sum(axis=0) * t1).sum())
        bal_sum += float(bl.sum())
    total = (mse_sum / C + ALPHA * bal_sum) / (B * T)
    return np.float32(total)


def kernel(preds, targets, nee_qc, igbp, koppen, igbp_table, koppen_table):
    for misc in (224, 1024, 4096):
        try:
            in_maps = make_in_maps(preds, targets, nee_qc, igbp, koppen,
                                   igbp_table, koppen_table, misc=misc)
        except OverflowError:
            continue
        res = _run_spmd(in_maps, misc)
        return finish(res, igbp_table, koppen_table)
    raise RuntimeError("bucket spill exceeded all misc capacities")
